# revision 47
# baseline (speedup 1.0000x reference)
"""DTSemNet forward (nn_DTSemNet_54528904790526) on 8 TRN2 NeuronCores.

Math: the reference computes
    x = in_x @ W1.T + b1                       [B, 2047]
    h = [relu(x), relu(-x)]                    [B, 4094]
    z = h @ L.T                                [B, 2048]   (frozen 0/1 leaf routing)
    out[b, a] = max over leaves ell with (ell % 10 == a) of z[b, ell]

L is the complete-binary-tree path matrix, so
    z[b, ell] = sum_i |x_i|  -  sum_{path nodes} penalty,
    penalty   = relu(-x_node) going left, relu(x_node) going right,
replacing the dense [B,4094]x[4094,2048] matmul with an 11-level tree DP
on the vector engine (see kernel_fp32r_baseline.py for the derivation).

This version runs the linear1 matmul in fp8 (e4m3) with
MatmulPerfMode.DoubleRowSwInterleave: 2 fp8 weights per PE cell ->
256-deep contraction per instruction; the software-interleaved
(pair-interleaved, column-reversed) stationary layout loads weights
contiguously, HW-measured ~1.4x faster per tile than plain DoubleRow
and ~2.8x faster than the fp32r baseline matmul. Inputs are quantized
host-side: activations cast straight to e4m3 (|x|<6 << 240, TRN e4m3
infs at 256), weights and bias pre-scaled by 2^7 (exact) to clear the
e4m3 subnormal floor at 2^-6; the relu undoes the scale exactly via the
activation scale (+-2^-7). Measured end-to-end error: max rel 8.1e-3 vs
the 2e-2 gate (matmul accumulates in fp32 PSUM; e4m3 products are exact
in fp32).

Per-core shard: batch rows (data parallel over 8 cores, 2048 rows each).
Per 128-row batch tile: 8 SwInterleave k-steps x 4 chunk matmuls
accumulate x*2^7 into one [128,2048] PSUM tile (4 banks); the bias row
is folded in as a K=1 ones-row DoubleRow matmul per chunk at 4
concurrent PE row-group positions. Two full-width activations produce
pen = relu(+-x) bf16 with accumulated row-sums (S_abs). The 11-level
tree DP runs on DVE in bf16 (the parent-broadcast AP blocks 2x packing;
GPSIMD measured ~4x slower than DVE on these APs, so it only carries
the output DMAs); the group-min is one packed TT-min pre-fold + one 4-D
strided reduce + a leftover fixup, with the DVE tail software-pipelined
one tile behind the DP. Pool ring depths are deliberately SHALLOW
(pen=2/dp=3/sm=4): deeper rings measured ~5 us slower (more ring-
semaphore traffic), xt prefetch stays at 6.

Measured (reliable 512-iteration repeat-loop deltas, single core, this
container): full ~146 us/core-sweep vs 248.6 us for the fp32r baseline
measured identically. Decomposition: matmul+DMA floor ~122-127 us, +ACT
~3 us, +DP ~13 us, +group-min ~3.5 us. The matmul floor is set by the
toolchain emitting one LDWEIGHTS per matmul (--enable-ldw-opt=false is
hardcoded; the 4 chunk matmuls re-load their shared stationary, ~107ns
each serialized with the ~107ns MM stream); wider 1024-col chunks that
would halve the LDW count are rejected by walrus codegen.
"""
import sys

sys.path.insert(0, "/opt/trn_rl_repo")
from contextlib import ExitStack

import numpy as np
import ml_dtypes

import concourse.bass as bass
import concourse.tile as tile
from concourse import bacc, mybir
from concourse.bass_utils import run_bass_kernel_spmd

# problem shape (hardcoded per contract)
B = 16384
D = 2048
N = 2047          # internal nodes
NP = 2048         # N padded (zero column 2047)
HEIGHT = 11
NL = 2048         # leaves
OUT = 10
NCORES = 8
BC = B // NCORES  # batch rows per core (2048)
KT = D // 128     # 16 k-subtiles of 128 contraction rows
KT2 = KT // 2     # 8 DoubleRow k-steps of 256
BT = BC // 128    # 16 batch tiles per core
CHUNKS = [(0, 512), (512, 1024), (1024, 1536), (1536, 2048)]
WSCALE = 128.0    # weight pre-scale 2^7 (exact in fp8/fp32)

F8 = ml_dtypes.float8_e4m3   # TRN fp8e4: inf at S.1111.000, max normal 240

f32 = mybir.dt.float32
fp8 = mybir.dt.float8e4
bf16 = mybir.dt.bfloat16
ADD = mybir.AluOpType.add
MIN = mybir.AluOpType.min
SUB = mybir.AluOpType.subtract
MULT = mybir.AluOpType.mult
RELU = mybir.ActivationFunctionType.Relu
AXX = mybir.AxisListType.X
DR = mybir.MatmulPerfMode.DoubleRow
SW = mybir.MatmulPerfMode.DoubleRowSwInterleave


def build_kernel(bt=BT, reps=1, loop_reps=None, mode="full"):
    """bt: number of batch tiles (128 rows each) this kernel processes.
    reps: python-unrolled repeats of the whole per-tile pipeline.
    loop_reps: device-side For_i repeats (for timing probes).
    mode: "full" | "nodp" (skip tree DP/mins) | "mmonly" (matmuls only)
          | "dponly" (memset pen, DP chain only)."""
    nc = bacc.Bacc("TRN2")
    # in_x fp8 shard, pre-blocked host-side as [bt][128 p][KT j][128 m]
    # with p = contraction row % 128, j = contraction row // 128, so each
    # SBUF partition reads one contiguous 2KB run per batch tile.
    xt = nc.dram_tensor("xt", [bt * 128, KT * 128], fp8, kind="ExternalInput")
    # W1.T * 2^7 (zero col at node 2047), blocked [128 p][KT j][NP n]
    wt = nc.dram_tensor("wt", [128, KT * NP], fp8, kind="ExternalInput")
    # ones rows for the bias matmul: [1.0]*128 | [0.0]*128 per row
    ones = nc.dram_tensor("ones", [4, 2 * 128], fp8, kind="ExternalInput")
    # bias rows: b1 * 2^7 (2048, zero-padded) | zeros(2048) per row
    wtb = nc.dram_tensor("wtb", [4, 2 * NP], fp8, kind="ExternalInput")
    out = nc.dram_tensor("out", [bt * 128, OUT], f32, kind="ExternalOutput")

    with tile.TileContext(nc) as tc, ExitStack() as ctx:
        wt_pool = ctx.enter_context(tc.tile_pool(name="wt", bufs=1))
        xt_pool = ctx.enter_context(tc.tile_pool(name="xt", bufs=6))
        ps_pool = ctx.enter_context(tc.tile_pool(name="ps", bufs=2, space="PSUM"))
        pen_pool = ctx.enter_context(tc.tile_pool(name="pen", bufs=2))
        dp_pool = ctx.enter_context(tc.tile_pool(name="dp", bufs=3))
        sm_pool = ctx.enter_context(tc.tile_pool(name="sm", bufs=4))

        # resident weights: [128, KT, NP] fp8 (32KB/partition)
        wts = wt_pool.tile([128, KT * NP], fp8, tag="wt")
        nc.sync.dma_start(wts[:], wt[:, :])
        wt3 = wts[:].rearrange("p (j n) -> p j n", n=NP)
        # bias + ones rows replicated at partitions 0/32/64/96 so the four
        # K=1 bias matmuls can run as concurrent PE row-group tiles
        ones_t = wt_pool.tile([128, 2 * 128], fp8, tag="ones")
        wtb_t = wt_pool.tile([128, 2 * NP], fp8, tag="wtb")
        for rg in range(4):
            nc.sync.dma_start(ones_t[32 * rg:32 * rg + 1, :], ones[rg:rg + 1, :])
            nc.sync.dma_start(wtb_t[32 * rg:32 * rg + 1, :], wtb[rg:rg + 1, :])
        ones3 = ones_t[:].rearrange("p (i m) -> p i m", m=128)
        wtb3 = wtb_t[:].rearrange("p (i n) -> p i n", n=NP)

        def body():
            pending = []
            for t in range(bt):
                c_lo = t * 128
                c_hi = (t + 1) * 128
                xt_t = xt_pool.tile([128, KT * 128], fp8, tag="xt")
                nc.sync.dma_start(xt_t[:], xt[c_lo:c_hi, :])
                # SwInterleave stationary layout: per k-step, pairs
                # (sub0, sub1) interleaved per batch column, columns reversed
                x3 = xt_t[:].rearrange("p (j t i) -> p j t i", i=2, t=128)

                pen = pen_pool.tile([128, 2 * NP], bf16, tag="pen")
                sacc = sm_pool.tile([128, 4], f32, tag="sacc")

                if mode == "dponly":
                    # fill pen/sacc cheaply so the DP chain is isolated
                    nc.vector.memset(pen[:], 0.5)
                    nc.vector.memset(sacc[:], 1.0)
                else:
                    ps = ps_pool.tile([128, NP], f32, tag="ps")
                    if mode == "mmnodma":
                        nc.vector.memset(xt_t[:], 0.25)
                    # k-outer order: 4 consecutive matmuls share the
                    # stationary xt block
                    for k in range(KT2):
                        for ci, (c0, c1) in enumerate(CHUNKS):
                            nc.tensor.matmul(
                                ps[:, c0:c1],
                                x3[:, k],
                                wt3[:, 2 * k:2 * k + 2, c0:c1],
                                start=(k == 0),
                                stop=(mode == "mmnob" and k == KT2 - 1),
                                perf_mode=SW,
                            )
                    if mode != "mmnob":
                        for ci, (c0, c1) in enumerate(CHUNKS):
                            bp = 32 * ci
                            nc.tensor.matmul(
                                ps[:, c0:c1],
                                ones3[bp:bp + 1, :, :],
                                wtb3[bp:bp + 1, :, c0:c1],
                                start=False, stop=True,
                                perf_mode=DR,
                                tile_position=(bp, 0),
                            )
                    if mode == "mmpure":
                        continue
                    if mode in ("mmonly", "mmnob", "mmnodma"):
                        outsb = sm_pool.tile([128, OUT], f32, tag="outsb")
                        nc.scalar.copy(outsb[:], ps[:, 0:OUT])
                        nc.sync.dma_start(out[c_lo:c_hi, :], outsb[:])
                        continue
                    # pen = [relu(-x) | relu(x)] with running row-sums;
                    # the 2^-7 scale undoes the weight pre-scale exactly.
                    # (Half-width ACT splits to start the DP earlier measured
                    # 30 us WORSE -- per-instruction ACT overhead dominates.)
                    nc.scalar.activation(
                        pen[:, NP:2 * NP], ps[:, 0:NP], RELU,
                        scale=1.0 / WSCALE,
                        accum_out=sacc[:, 0:1],
                    )
                    nc.scalar.activation(
                        pen[:, 0:NP], ps[:, 0:NP], RELU,
                        scale=-1.0 / WSCALE,
                        accum_out=sacc[:, 1:2],
                    )

                if mode in ("nodp", "nomin"):
                    sabs = sm_pool.tile([128, 1], f32, tag="sabs")
                    nc.vector.tensor_tensor(
                        sabs[:, 0:1], sacc[:, 0:1], sacc[:, 1:2], op=ADD)
                if mode == "nodp":
                    outsb = sm_pool.tile([128, OUT], f32, tag="outsb")
                    nc.scalar.copy(outsb[:, 0:1], sabs[:])
                    nc.sync.dma_start(out[c_lo:c_hi, 0:1], outsb[:, 0:1])
                    continue

                # ---- tree DP over 11 levels, split (evens|odds) layout ----
                # One TT per level: out[s, j, u] = par[j, u] + pen[s][n0 + 2j+u]
                # (s = 0 left / 1 right half; parent broadcast via stride-0 dim)
                # level-1 costs are pen[0] (left child) and pen[NP] (right)
                par = pen[:, 0:2 * NP:NP].rearrange("p (j u) -> p j u", u=2)
                pen2 = pen.rearrange("p (s c) -> p s c", s=2)
                lvl = None
                for d in range(1, HEIGHT):
                    w = 1 << d          # number of level-d nodes = parents
                    n0 = w - 1          # first node index of level d
                    nxt = dp_pool.tile([128, 2 * w], bf16, tag=f"lvl{d + 1}")
                    out4 = nxt[:].rearrange("p (s j u) -> p s j u", s=2, u=2)
                    pen4 = pen2[:, :, n0:n0 + w].rearrange(
                        "p s (j u) -> p s j u", u=2)
                    par4 = par.rearrange("p (x j) u -> p x j u", x=1)
                    par4 = par4.broadcast_to([128, 2, w // 2, 2])
                    nc.vector.tensor_tensor(out4, par4, pen4, op=ADD)
                    lvl = nxt
                    par = nxt[:].rearrange("p (u j) -> p j u", u=2)

                if mode == "nomin":
                    outsb = sm_pool.tile([128, OUT], f32, tag="outsb")
                    nc.vector.tensor_scalar(
                        outsb[:], lvl[:, 0:OUT], sabs[:, 0:1], -1.0,
                        op0=SUB, op1=MULT,
                    )
                    nc.sync.dma_start(out[c_lo:c_hi, :], outsb[:])
                    continue

                # lvl holds leaf costs [128, 2048]: evens | odds halves.
                # group a=2r+s: min over positions m ≡ r (mod 5) of half s.
                # Pre-fold 1020 -> 510 per half (offset 510 ≡ 0 mod 5 keeps
                # residues) on GPSIMD via min(a,b) = a - relu(a-b) (Pool has
                # no TT-min); then on DVE ONE 4-D strided reduce
                # [s][r=5][j=102] -> tmp[s*5+r], a fused 4-elem leftover
                # fixup, and the final subtract. The DVE tail is emitted one
                # tile late (software pipelining) so the DVE FIFO never
                # head-of-line-blocks on the Pool folds.
                lvl2 = lvl[:].rearrange("p (s c) -> p s c", s=2)
                fold = dp_pool.tile([128, 2 * 512], bf16, tag="fold")
                fold2 = fold[:].rearrange("p (s c) -> p s c", s=2)
                nc.vector.tensor_tensor(
                    fold2[:, :, 0:510], lvl2[:, :, 0:510],
                    lvl2[:, :, 510:1020], op=MIN,
                )
                # leftover m=1020..1023 (residues 0..3) fold into the
                # matching residue slots BEFORE the reduce, so the reduce
                # can negate its output and the final op becomes
                # (-min + sacc0) + sacc1 -- no separate S_abs sum needed
                nc.vector.tensor_tensor(
                    fold2[:, :, 0:4], fold2[:, :, 0:4],
                    lvl2[:, :, 1020:1024], op=MIN,
                )

                def min_tail(fold2=fold2, sacc=sacc, c_lo=c_lo, c_hi=c_hi):
                    tmp = sm_pool.tile([128, 2 * 5], bf16, tag="mins")
                    tmp3 = tmp[:].rearrange("p (s r) -> p s r", s=2)
                    src = fold2[:, :, 0:510].rearrange(
                        "p s (j r) -> p s r j", r=5)
                    nc.vector.tensor_reduce(
                        tmp3, src, axis=AXX, op=MIN, negate=True)
                    outsb = sm_pool.tile([128, OUT], f32, tag="outsb")
                    # out[:, 2r+s] = (-min[s*5+r] + sacc0) + sacc1
                    out_perm = outsb[:].rearrange("p (r s) -> p s r", s=2)
                    nc.vector.tensor_scalar(
                        out_perm, tmp3, sacc[:, 0:1], sacc[:, 1:2],
                        op0=ADD, op1=ADD,
                    )
                    # out DMA via the idle Pool SWDGE: it depends on the late
                    # DVE tail, and on the SP queue it would head-of-line
                    # block the next tiles' xt input loads
                    nc.gpsimd.dma_start(out[c_lo:c_hi, :], outsb[:])

                pending.append(min_tail)
                if len(pending) > 1:
                    pending.pop(0)()
            for fn in pending:
                fn()

        if loop_reps is not None:
            with tc.For_i(0, loop_reps):
                body()
        else:
            for _ in range(reps):
                body()

    nc.finalize()
    return nc


_NC_CACHE = {}


def _get_nc():
    key = (BT, 1)
    if key not in _NC_CACHE:
        _NC_CACHE[key] = build_kernel()
    return _NC_CACHE[key]


def marshal_xt(in_x_shard):
    """[BC, D] f32 rows -> fp8 [BT*128, KT*128] blocked for the
    DoubleRowSwInterleave stationary layout: SBUF partition p of batch
    tile t holds, per k-step j, 128 interleaved pairs in reversed batch
    order: out[t*128+p, j*256 + 2*q + i] = in_x_shard[t*128 + (127-q),
    j*256 + i*128 + p]."""
    bt = in_x_shard.shape[0] // 128
    a = in_x_shard.reshape(bt, 128, KT2, 2, 128)    # [t, m, j, i, p]
    a = a[:, ::-1]                                  # m -> q = 127-m
    a = a.transpose(0, 4, 2, 1, 3).astype(F8)       # [t, p, j, q, i]
    return np.ascontiguousarray(a.reshape(bt * 128, KT * 128))


def _weight_maps(W1, b1):
    # wt[p, j*NP + n] = W1[n, j*128 + p] * 2^7 (node 2047 column = 0)
    wtf = np.zeros((D, NP), np.float32)
    wtf[:, :N] = W1.T * WSCALE
    wq = wtf.reshape(KT, 128, NP).transpose(1, 0, 2).astype(F8)  # [p, j, n]
    wq = np.ascontiguousarray(wq.reshape(128, KT * NP))
    onesq = np.zeros((4, 2 * 128), np.float32)
    onesq[:, 0:128] = 1.0
    wtbq = np.zeros((4, 2 * NP), np.float32)
    wtbq[:, :N] = b1 * WSCALE
    return wq, onesq.astype(F8), wtbq.astype(F8)


def make_in_map(in_x_shard, W1, b1):
    wq, onesq, wtbq = _weight_maps(W1, b1)
    return {"xt": marshal_xt(np.asarray(in_x_shard, np.float32)),
            "wt": wq, "ones": onesq, "wtb": wtbq}


def kernel(in_x, W1, b1, L, A):
    in_x = np.asarray(in_x, np.float32)
    W1 = np.asarray(W1, np.float32)
    b1 = np.asarray(b1, np.float32)
    wq, onesq, wtbq = _weight_maps(W1, b1)
    in_maps = [
        {"xt": marshal_xt(in_x[c * BC:(c + 1) * BC]), "wt": wq,
         "ones": onesq, "wtb": wtbq}
        for c in range(NCORES)
    ]
    nc = _get_nc()
    res = run_bass_kernel_spmd(nc, in_maps, core_ids=list(range(NCORES)))
    return np.concatenate([res.results[c]["out"] for c in range(NCORES)], axis=0)
